# Initial kernel scaffold
#
"""Single-head causal attention, distributed across 8 TRN2 NeuronCores.

Reference computation (fp32):
    Q = x @ Wq.T; K = x @ Wk.T; V = x @ Wv.T        # x [B=4, T=4096, C=768], W* [H=64, C]
    out = softmax(causal(Q @ K.T / sqrt(C))) @ V     # out [B, T, H]

Sharding: 8 cores = 4 batches x 2 query-halves. Core c handles batch b=c//2,
query rows [p*2048, (p+1)*2048) with p=c%2. Each core receives xT [768, 4096]
(pre-transposed on host): columns [0:2048) = "context" rows (batch rows
[0:2048), zeros for p=0 since it has no context), columns [2048:4096) = the
core's own 2048 rows. SPMD-uniform program; the only per-core variation is
data: a gate bias (0.0 visible / -50.0 masked) folded into the exp() so p=0
cores numerically kill their context block (exp(s-50)*2048 ~ 4e-19).

Kernel layout: scores are computed transposed, St[k_par, q_free] =
matmul(lhsT=Kt[:,kb], rhs=Qt), so softmax' P tiles feed the P@V matmul
directly as the moving operand with V|ones [k_par, 65] stationary; the
appended ones column accumulates the softmax denominator in the same PSUM
accumulation. Causal structure inside the own block is identical on all
cores => static per-diagonal masks; k-tiles fully above the diagonal are
statically skipped. All matmul I/O is float32r (fp32 bits, ~tf32 precision,
full-rate PE streaming).
"""

import numpy as np

B, T, C, H = 4, 4096, 768, 64
TQ = 2048          # own query rows per core
NKC = 8            # 512-wide column chunks of xT
NQC = 4            # 512-wide query chunks
NKB = 32           # 128-wide k tiles
CTXB = 16          # k tiles in the context region
SCALE = float(C) ** -0.5

DEFAULT_CFG = dict(
    pad_st=False,    # pad St contract to 128 (zeros in Kt rows 64:128)
    stp_bufs=3,      # PSUM buffers for score tiles
    pexp_bufs=1,     # SBUF buffers per staged P tile tag
    ptr_sep=True,    # small transpose PSUM tiles in their own tag
    stp_cols=512,    # score-tile width (exp instruction granularity)
    structure="split",  # "split" | "inter"
    pv_stop_each=False,  # close the PV accumulation group after every matmul
    pv_banks=1,      # number of PV PSUM accumulators (summed at finalize)
    exp_x=1,         # emit exp N times (marginal-cost probe)
    st_x=1,          # emit each St matmul N times (marginal-cost probe)
)


def build_bass(niter: int = 1, ablate: frozenset = frozenset(), **cfg_over):
    import concourse.bacc as bacc
    import concourse.mybir as mybir
    from concourse import tile
    from concourse.masks import make_identity

    cfg = dict(DEFAULT_CFG)
    cfg.update(cfg_over)
    pad = cfg["pad_st"]
    stp_cols = cfg["stp_cols"]
    nhalf = stp_cols // 512
    ptr_tag = "ptr" if cfg["ptr_sep"] else "pvq"

    fp32 = mybir.dt.float32
    f32r = mybir.dt.float32r
    Exp = mybir.ActivationFunctionType.Exp

    nc = bacc.Bacc("TRN2", target_bir_lowering=False, num_devices=8)
    xT_d = nc.dram_tensor("xT", [C, T], fp32, kind="ExternalInput")
    wT_d = nc.dram_tensor("wT", [C, 4 * H], fp32, kind="ExternalInput")
    gate_d = nc.dram_tensor("gate", [128, 1], fp32, kind="ExternalInput")
    out_d = nc.dram_tensor("out", [TQ, H], fp32, kind="ExternalOutput")

    with tile.TileContext(nc) as tc:
        with (
            tc.tile_pool(name="const", bufs=1) as constp,
            tc.tile_pool(name="data", bufs=1) as datap,
            tc.tile_pool(name="work", bufs=3) as workp,
            tc.tile_pool(name="ps", bufs=2, space="PSUM") as psp,
        ):
            def body(_iv=None):
                w_sb = constp.tile([128, 6, 4 * H], f32r, tag="w")
                nc.sync.dma_start(
                    w_sb[:], wT_d.ap().rearrange("(a p) n -> p a n", p=128).bitcast(f32r)
                )
                gate_sb = constp.tile([128, 1], fp32, tag="gate")
                nc.sync.dma_start(gate_sb[:], gate_d.ap())
                id_sb = constp.tile([128, 128], fp32, tag="id")
                make_identity(nc, id_sb[:])

                # 4 diagonal masks: dmask[d][pi, fj] = 1.0 iff fj - 128*d - pi >= 0
                dmasks = []
                for d in range(4):
                    dm = constp.tile([128, 512], fp32, tag=f"dmask{d}")
                    nc.gpsimd.memset(dm[:], 1.0)
                    nc.gpsimd.affine_select(
                        out=dm[:], in_=dm[:],
                        compare_op=mybir.AluOpType.is_ge,
                        fill=0.0, base=-128 * d,
                        channel_multiplier=-1, pattern=[[1, 512]],
                    )
                    dmasks.append(dm)

                kt_sb = datap.tile([128, T], f32r, tag="kt")
                if pad:
                    nc.vector.memset(kt_sb[64:128, :].bitcast(fp32), 0.0)
                qt_sb = datap.tile([128, TQ], f32r, tag="qt")
                vones = datap.tile([128, NKB, H + 1], f32r, tag="vones")
                nc.vector.memset(vones[:, :, H : H + 1].bitcast(fp32), 1.0)

                xts = []
                xtp_ctx = tc.tile_pool(name="xtp", bufs=1)
                xtp = xtp_ctx.__enter__()
                for kc in range(NKC):
                    xt = xtp.tile([128, 6, 512], f32r, tag=f"xt{kc}")
                    if "dma" not in ablate:
                        nc.sync.dma_start(
                            xt[:],
                            xT_d.ap()
                            .rearrange("(a p) n -> p a n", p=128)[
                                :, :, 512 * kc : 512 * (kc + 1)
                            ].bitcast(f32r),
                        )
                    if "dma" in ablate:
                        nc.vector.memset(xt[:, :, 0:1].bitcast(fp32), 0.5)
                    xts.append(xt)

                # K^T and V projections, 512 columns at a time.
                for kc in range(0 if "proj" not in ablate else NKC, NKC):
                    pskv = psp.tile([128, 512], fp32, tag="stkv",
                                    bufs=cfg["stp_bufs"])
                    for ct in range(6):
                        nc.tensor.matmul(
                            pskv[:],
                            lhsT=w_sb[:, ct, 2 * H : 4 * H],
                            rhs=xts[kc][:, ct, :],
                            start=(ct == 0), stop=(ct == 5),
                        )
                    nc.vector.tensor_copy(
                        kt_sb[0:64, 512 * kc : 512 * (kc + 1)], pskv[0:64, :]
                    )
                    vt_sb = workp.tile([64, 512], fp32, tag="vt")
                    nc.vector.tensor_copy(vt_sb[:], pskv[64:128, :])
                    for j in range(4):
                        kb = 4 * kc + j
                        pst = psp.tile([128, H], fp32, tag=ptr_tag, bufs=2)
                        nc.tensor.transpose(
                            pst[:], vt_sb[:, 128 * j : 128 * (j + 1)], id_sb[0:64, 0:64]
                        )
                        nc.vector.tensor_copy(vones[:, kb, 0:H], pst[:])

                # Q^T projection (Wq pre-scaled by 1/sqrt(C) on host;
                # packed twice so rows 64:127 duplicate rows 0:63).
                for qc in range(0 if "proj" not in ablate else NQC, NQC):
                    psq = psp.tile([128, 512], fp32, tag="pvq", bufs=2)
                    for ct in range(6):
                        nc.tensor.matmul(
                            psq[:],
                            lhsT=w_sb[:, ct, 0 : 2 * H],
                            rhs=xts[4 + qc][:, ct, :],
                            start=(ct == 0), stop=(ct == 5),
                        )
                    nc.scalar.copy(qt_sb[:, 512 * qc : 512 * (qc + 1)], psq[:])

                xtp_ctx.__exit__(None, None, None)
                pep_ctx = tc.tile_pool(name="pep", bufs=1)
                pep = pep_ctx.__enter__()

                if "proj" in ablate:
                    nc.vector.memset(kt_sb[:].bitcast(fp32), 0.5)
                    nc.vector.memset(qt_sb[:].bitcast(fp32), 0.5)
                    nc.vector.memset(vones[:, :, 0:H].bitcast(fp32), 0.5)

                krows = slice(0, 128) if pad else slice(0, 64)

                # Attention: St = Kt_blk.T @ Qt_chunk -> exp -> mask -> PV.
                nbank = cfg["pv_banks"]
                inter = cfg["structure"] == "inter"
                for qc in range(NQC):
                    n_kb = CTXB + 4 * qc + 4
                    ngrp = n_kb // nhalf
                    pvs_banks = [
                        psp.tile([H + 1, 512], fp32, tag="pvq", bufs=2,
                                 name=f"pv{i}")
                        for i in range(nbank)
                    ]
                    if "pv" in ablate:
                        for pvb in pvs_banks:
                            nc.vector.memset(pvb[:], 1.0)
                    pexps = []
                    started = [False] * nbank

                    def pv_mm(pr, half, pexp):
                        if "pv" in ablate:
                            return
                        kb = nhalf * pr + half
                        i = (nhalf * pr + half) % nbank
                        last = pr == ngrp - 1 and half == nhalf - 1
                        nc.tensor.matmul(
                            pvs_banks[i][:],
                            lhsT=vones[:, kb, :],
                            rhs=pexp[:, 512 * half : 512 * (half + 1)],
                            start=not started[i],
                            stop=(cfg["pv_stop_each"] or last
                                  or (inter and nbank > 1)),
                            skip_group_check=True,
                        )
                        started[i] = True

                    for pr in range(ngrp):
                        stp = psp.tile([128, stp_cols], fp32, tag="stkv",
                                       bufs=cfg["stp_bufs"])
                        pexp = pep.tile([128, stp_cols], f32r,
                                        tag=f"pexp{pr if not inter else pr % 4}",
                                        bufs=cfg["pexp_bufs"])
                        if "st" in ablate:
                            nc.vector.memset(stp[:, 0:1], 0.1)
                        for half in range(nhalf if "st" not in ablate else 0):
                            kb = nhalf * pr + half
                            for _x in range(cfg["st_x"]):
                                nc.tensor.matmul(
                                    stp[:, 512 * half : 512 * (half + 1)],
                                    lhsT=kt_sb[krows, 128 * kb : 128 * (kb + 1)],
                                    rhs=qt_sb[krows, 512 * qc : 512 * (qc + 1)],
                                    start=True, stop=True, skip_group_check=True,
                                )
                        if "exp" in ablate:
                            nc.vector.memset(pexp[:].bitcast(fp32), 0.5)
                        if "exp" not in ablate:
                            for _x in range(cfg["exp_x"]):
                                nc.scalar.activation(
                                    pexp[:], stp[:], Exp,
                                    bias=(gate_sb[:] if (nhalf * pr + nhalf - 1) < CTXB
                                          else 0.0),
                                )
                        for half in range(nhalf):
                            kb = nhalf * pr + half
                            b = kb - CTXB
                            if 4 * qc <= b <= 4 * qc + 3 and "mask" not in ablate:
                                nc.vector.tensor_mul(
                                    pexp[:, 512 * half : 512 * (half + 1)],
                                    pexp[:, 512 * half : 512 * (half + 1)],
                                    dmasks[b - 4 * qc][:],
                                )
                            if inter:
                                pv_mm(pr, half, pexp)
                        pexps.append(pexp)
                    if not inter:
                        for pr in range(ngrp):
                            for half in range(nhalf):
                                pv_mm(pr, half, pexps[pr])
                    pv = pvs_banks[0]
                    for pvb in pvs_banks[1:]:
                        nc.vector.tensor_add(pv[:], pv[:], pvb[:])
                    # normalize + transpose back to [q, h]
                    pvs = workp.tile([H + 1, 512], fp32, tag="pvs")
                    nc.vector.tensor_copy(pvs[:], pv[:])
                    for j in range(4):
                        pst2 = psp.tile([128, H + 1], fp32, tag=ptr_tag, bufs=2)
                        nc.tensor.transpose(
                            pst2[:],
                            pvs[:, 128 * j : 128 * (j + 1)],
                            id_sb[0 : H + 1, 0 : H + 1],
                        )
                        rec = workp.tile([128, 1], fp32, tag="rec")
                        nc.vector.reciprocal(rec[:], pst2[:, H : H + 1])
                        outt = workp.tile([128, H], fp32, tag="outt")
                        nc.vector.tensor_scalar_mul(outt[:], pst2[:, 0:H], rec[:])
                        r0 = 512 * qc + 128 * j
                        nc.sync.dma_start(out_d.ap()[r0 : r0 + 128, :], outt[:])
                pep_ctx.__exit__(None, None, None)

            if niter == 1:
                body()
            else:
                with tc.For_i(0, niter) as iv:
                    body(iv)

    nc.compile()
    return nc


_NC_CACHE = {}


def _get_nc(niter: int = 1):
    if niter not in _NC_CACHE:
        _NC_CACHE[niter] = build_bass(niter)
    return _NC_CACHE[niter]


def make_in_maps(x, Wq, Wk, Wv):
    wqs = Wq.T * SCALE
    wT = np.concatenate([wqs, wqs, Wk.T, Wv.T], axis=1).astype(np.float32)
    wT = np.ascontiguousarray(wT)
    in_maps = []
    for c in range(8):
        b, p = c // 2, c % 2
        xT = np.zeros((C, T), np.float32)
        if p == 1:
            xT[:, 0:TQ] = x[b, 0:TQ, :].T
        xT[:, TQ:T] = x[b, p * TQ : (p + 1) * TQ, :].T
        gate = np.full((128, 1), 0.0 if p == 1 else -50.0, np.float32)
        in_maps.append(
            {"xT": np.ascontiguousarray(xT), "wT": wT, "gate": gate}
        )
    return in_maps


def kernel(x, Wq, Wk, Wv):
    from concourse.bass_utils import run_bass_kernel_spmd

    x = np.asarray(x, np.float32)
    nc = _get_nc(1)
    in_maps = make_in_maps(x, np.asarray(Wq), np.asarray(Wk), np.asarray(Wv))
    res = run_bass_kernel_spmd(nc, in_maps, core_ids=list(range(8)), trace=False)
    out = np.empty((B, T, H), np.float32)
    for c in range(8):
        b, p = c // 2, c % 2
        out[b, p * TQ : (p + 1) * TQ, :] = res.results[c]["out"]
    return out



# revision 3
# speedup vs baseline: 1.2117x; 1.2117x over previous
"""Single-head causal attention, distributed across 8 TRN2 NeuronCores.

Reference (fp32):
    Q = x @ Wq.T; K = x @ Wk.T; V = x @ Wv.T        # x [B=4, T=4096, C=768], W* [H=64, C]
    out = softmax(causal(Q @ K.T / sqrt(C))) @ V     # out [B, T, H]

Sharding: 8 cores = 4 batches x 2 query-interleaves. Core c: batch b=c//2,
p=c%2 takes 512-row query chunks {2m+p : m=0..3} — balanced causal work.
SPMD-uniform program; per-core variation is data only (see make_in_maps):
xT sub-block swap per pair, compile-time diagonal masks, per-core 0/1
kill scalar for the final pair's partner sub-block.

Performance structure (HW-measured drivers):
  * x arrives pre-swizzled so the whole tensor is per-partition contiguous;
    loaded in cfg.dma_split large DMAs alternating the two HWDGE rings
    (sync/scalar) — large transfers run at ~3x the rate of 786 KB slices.
  * outputs staged in SBUF, one DMA per evaluation (16 small DMAs were
    descriptor-dominated).
  * St runs as 2 concurrent K=64 matmuls in disjoint PE row-groups
    (tile_position via base_partition 0/64) — kt/qt rows 64:128 duplicate
    rows 0:64 to feed the second row-group.
  * cfg.ubody evaluations per For_i iteration pipeline against each other
    (For_i ends in an all-engine barrier, so cross-iteration overlap is
    impossible; in-loop unrolling recovers it).
All SBUF data bf16; PSUM fp32; exp without max-subtraction (scores ~N(0,0.01));
softmax denominator rides as a 65th PV row (vones = [V | 1]).
"""

import numpy as np

B, T, C, H = 4, 4096, 768, 64
TQ = 2048          # own query rows per core
CH = 512           # query chunk width
NCH = 4            # chunks per core
NKB = 32           # 128-wide key tiles
SCALE = float(C) ** -0.5

DEFAULT_CFG = dict(
    big_bufs=2,      # PSUM buffers for [128,1024] score/proj tiles
    pexp_bufs=4,     # SBUF buffers for staged P tiles
    staggered=True,
    rowtile=True,
    ubody=2,         # evaluations per For_i iteration (pipelined)
    data_bufs=2,     # double-buffer the data pool across evaluations
    dma_split=8,     # number of x DMA slices (contiguous per partition)
    pv_lag=True,     # emit PV one group late so St(g+1) precedes PV(g) on PE
    dual_ring=False, # sync-ring only (scalar-ring DMA measured slower)
)


def build_bass(niter: int = 1, **cfg_over):
    import concourse.bacc as bacc
    import concourse.mybir as mybir
    from concourse import tile
    from concourse.masks import make_identity

    cfg = dict(DEFAULT_CFG)
    cfg.update(cfg_over)

    fp32 = mybir.dt.float32
    bf16 = mybir.dt.bfloat16
    Exp = mybir.ActivationFunctionType.Exp

    nc = bacc.Bacc("TRN2", target_bir_lowering=False, num_devices=8)
    # x^T pre-swizzled on host to [part 128, slice 8, ct 6, 512] flattened
    xp_d = nc.dram_tensor("xp", [128, 8 * 6 * 512], bf16, kind="ExternalInput")
    # w pre-swizzled to [part 128, ct 6, 3H]
    wp_d = nc.dram_tensor("wp", [128, 6 * 3 * H], bf16, kind="ExternalInput")
    g01_d = nc.dram_tensor("g01", [128, 1], fp32, kind="ExternalInput")
    # out[i, m, h] = row 128m+i of the core's 2048 rows
    out_d = nc.dram_tensor("out", [128, 16 * H], fp32, kind="ExternalOutput")

    with tile.TileContext(nc) as tc:
        with (
            tc.tile_pool(name="const", bufs=1) as constp,
            tc.tile_pool(name="data", bufs=cfg["data_bufs"]) as datap,
            tc.tile_pool(name="work", bufs=3) as workp,
            tc.tile_pool(name="pep", bufs=1) as pep,
            tc.tile_pool(name="ps", bufs=2, space="PSUM") as psp,
        ):
            def body(_iv=None):
                rt = cfg["rowtile"]
                w_sb = constp.tile([128, 6, 3 * H], bf16, tag="w")
                nc.sync.dma_start(
                    w_sb[:], wp_d.ap().rearrange("p (a n) -> p a n", a=6)
                )
                id_sb = constp.tile([128, 128], bf16, tag="id")
                make_identity(nc, id_sb[:])
                id32_sb = constp.tile([H + 1, H + 1], fp32, tag="id32")
                make_identity(nc, id32_sb[:])

                # diagonal masks for the own sub-block's 4 key tiles:
                # dm[i, s*512+qq] = 1.0 iff qq >= 128*s + i
                dm = constp.tile([128, 4, CH], bf16, tag="dm")
                nc.gpsimd.memset(dm[:], 1.0)
                nc.gpsimd.affine_select(
                    out=dm[:], in_=dm[:],
                    compare_op=mybir.AluOpType.is_ge,
                    fill=0.0, base=0,
                    channel_multiplier=-1, pattern=[[-128, 4], [1, CH]],
                )
                g01_sb = constp.tile([128, 1], fp32, tag="g01")
                nc.sync.dma_start(g01_sb[:], g01_d.ap())

                # rows 64:128 duplicate rows 0:64 when row-tiling
                kt = datap.tile([2 * H if rt else H, T], bf16, tag="kt")
                qt = datap.tile([2 * H if rt else H, TQ], bf16, tag="qt")
                vones = datap.tile([128, NKB, H + 1], bf16, tag="vones")
                nc.vector.memset(vones[:, :, H : H + 1], 1.0)
                # staged output, written back in one DMA
                outs = datap.tile([128, 16, H], fp32, tag="outs")

                # x^T in SBUF, slice-major to mirror DRAM: [part, slice 8,
                # ct 6, 512]; whole tensor contiguous per partition -> few
                # large line-rate DMAs
                xt = datap.tile([128, 8, 6, 512], bf16, tag="xt")
                xflat = xt[:].rearrange("p s a n -> p (s a n)")
                ns = cfg["dma_split"]
                step = (8 * 6 * 512) // ns
                for s in range(ns):
                    eng = nc.sync if (s % 2 == 0 or not cfg["dual_ring"]) \
                        else nc.scalar
                    eng.dma_start(
                        xflat[:, step * s : step * (s + 1)],
                        xp_d.ap()[:, step * s : step * (s + 1)],
                    )

                def kv_group(g):
                    # 1024 key-columns of K/V projection
                    c0 = 1024 * g
                    pskv = psp.tile([128, 1024], fp32, tag="proj", bufs=1)
                    for half in range(2):
                        for ct in range(6):
                            nc.tensor.matmul(
                                pskv[:, 512 * half : 512 * (half + 1)],
                                lhsT=w_sb[:, ct, H : 3 * H],
                                rhs=xt[:, 2 * g + half, ct, :],
                                start=(ct == 0), stop=(ct == 5),
                            )
                    nc.vector.tensor_copy(kt[0:H, c0 : c0 + 1024],
                                          pskv[0:64, :])
                    if rt:
                        nc.vector.tensor_copy(
                            kt[H : 2 * H, c0 : c0 + 1024],
                            kt[0:H, c0 : c0 + 1024],
                        )
                    vt = workp.tile([64, 1024], bf16, tag="vt")
                    nc.vector.tensor_copy(vt[:], pskv[64:128, :])
                    ptr = psp.tile([128, 544], bf16, tag="ptrv", bufs=1)
                    for mm in range(8):
                        nc.tensor.transpose(
                            ptr[:, 64 * mm : 64 * (mm + 1)],
                            vt[:, 128 * mm : 128 * (mm + 1)],
                            id_sb[0:64, 0:64],
                        )
                    t0 = 8 * g
                    nc.vector.tensor_copy(
                        vones[:, t0 : t0 + 8, 0:H], ptr[:, 0:512]
                    )

                def q_group(g2):
                    # query chunks {2g2, 2g2+1}: xt cols 2048g2 + {0,1024} + [0:512)
                    psq = psp.tile([128, 1024], fp32, tag="proj", bufs=1)
                    for half in range(2):
                        for ct in range(6):
                            nc.tensor.matmul(
                                psq[0:64, 512 * half : 512 * (half + 1)],
                                lhsT=w_sb[:, ct, 0:H],
                                rhs=xt[:, 4 * g2 + 2 * half, ct, :],
                                start=(ct == 0), stop=(ct == 5),
                            )
                    nc.vector.tensor_copy(
                        qt[0:H, 1024 * g2 : 1024 * (g2 + 1)], psq[0:64, :]
                    )
                    if rt:
                        nc.vector.tensor_copy(
                            qt[H : 2 * H, 1024 * g2 : 1024 * (g2 + 1)],
                            qt[0:H, 1024 * g2 : 1024 * (g2 + 1)],
                        )

                def att_chunk(m):
                    # chunk m: k-tiles 0..8m+7 in groups of 2 ([128,1024])
                    ntile = 8 * m + 8
                    ngrp = ntile // 2
                    pv = psp.tile([H + 1, CH], fp32, tag="pv", bufs=1)

                    def pv_group(g, pexp):
                        for u in range(2):
                            t = 2 * g + u
                            nc.tensor.matmul(
                                pv[:],
                                lhsT=vones[:, t, :],
                                rhs=pexp[:, 512 * u : 512 * (u + 1)],
                                start=(t == 0), stop=(t == ntile - 1),
                                skip_group_check=True,
                            )

                    lag = []
                    for g in range(ngrp):
                        stp = psp.tile([128, 1024], fp32, tag="big",
                                       bufs=cfg["big_bufs"])
                        for u in range(2):
                            t = 2 * g + u
                            r0 = H * u if rt else 0
                            nc.tensor.matmul(
                                stp[:, 512 * u : 512 * (u + 1)],
                                lhsT=kt[r0 : r0 + H, 128 * t : 128 * (t + 1)],
                                rhs=qt[r0 : r0 + H, CH * m : CH * (m + 1)],
                                start=True, stop=True, skip_group_check=True,
                            )
                        pexp = pep.tile([128, 1024], bf16, tag="pexp",
                                        bufs=cfg["pexp_bufs"])
                        nc.scalar.activation(pexp[:], stp[:], Exp)
                        if g == 4 * m or g == 4 * m + 1:
                            s0 = 2 * g - 8 * m
                            nc.vector.tensor_mul(
                                pexp[:],
                                pexp[:],
                                dm[:, s0 : s0 + 2, :]
                                .rearrange("p a b -> p (a b)"),
                            )
                        elif g >= 4 * m + 2:
                            nc.vector.tensor_scalar_mul(
                                pexp[:], pexp[:], g01_sb[:]
                            )
                        if cfg["pv_lag"]:
                            lag.append((g, pexp))
                            if len(lag) > 1:
                                pv_group(*lag.pop(0))
                        else:
                            pv_group(g, pexp)
                    for item in lag:
                        pv_group(*item)
                    # normalize + transpose back to [q, h] into the staging tile
                    pvs = workp.tile([H + 1, CH], fp32, tag="pvs")
                    nc.vector.tensor_copy(pvs[:], pv[:])
                    pstv = psp.tile([128, 544], bf16, tag="ptrv", bufs=1)
                    pst32 = pstv[:].bitcast(fp32)
                    for v in range(4):
                        nc.tensor.transpose(
                            pst32[:, 65 * v : 65 * v + H + 1],
                            pvs[:, 128 * v : 128 * (v + 1)],
                            id32_sb[:],
                        )
                        rec = workp.tile([128, 1], fp32, tag="rec")
                        nc.vector.reciprocal(
                            rec[:], pst32[:, 65 * v + H : 65 * v + H + 1]
                        )
                        nc.vector.tensor_scalar_mul(
                            outs[:, 4 * m + v, :],
                            pst32[:, 65 * v : 65 * v + H], rec[:]
                        )

                kv_group(0)
                q_group(0)
                att_chunk(0)
                kv_group(1)
                att_chunk(1)
                kv_group(2)
                q_group(1)
                att_chunk(2)
                kv_group(3)
                att_chunk(3)
                nc.sync.dma_start(
                    out_d.ap().rearrange("p (a n) -> p a n", a=16), outs[:]
                )

            if niter == 1:
                body()
            elif cfg.get("unroll"):
                for _ in range(niter):
                    body()
            else:
                with tc.For_i(0, niter,
                              staggered_reset=cfg.get("staggered", False)) as iv:
                    for _u in range(cfg["ubody"]):
                        body(iv)

    nc.compile()
    return nc


_NC_CACHE = {}


def _get_nc(niter: int = 1):
    if niter not in _NC_CACHE:
        _NC_CACHE[niter] = build_bass(niter)
    return _NC_CACHE[niter]


def make_in_maps(x, Wq, Wk, Wv):
    import ml_dtypes

    bf = ml_dtypes.bfloat16
    wT = np.concatenate([Wq.T * SCALE, Wk.T, Wv.T], axis=1).astype(bf)
    wp = np.ascontiguousarray(
        wT.reshape(6, 128, 3 * H).transpose(1, 0, 2).reshape(128, 6 * 3 * H)
    )
    in_maps = []
    for c in range(8):
        b, p = c // 2, c % 2
        xT = np.asarray(x[b].T, dtype=bf)          # [C, T] orig key order
        if p == 1:
            xT = xT.reshape(C, 4, 2, CH)[:, :, ::-1, :].reshape(C, T)
        # [C, T] -> [part, slice 8, ct 6, 512] flattened:
        # row r = ct*128 + part, col = slice*512 + n
        xp = np.ascontiguousarray(
            xT.reshape(6, 128, 8, 512).transpose(1, 2, 0, 3)
            .reshape(128, 8 * 6 * 512)
        )
        g01 = np.full((128, 1), 1.0 if p == 1 else 0.0, np.float32)
        in_maps.append({"xp": xp, "wp": wp, "g01": g01})
    return in_maps


def kernel(x, Wq, Wk, Wv):
    from concourse.bass_utils import run_bass_kernel_spmd

    x = np.asarray(x, np.float32)
    nc = _get_nc(1)
    in_maps = make_in_maps(x, np.asarray(Wq), np.asarray(Wk), np.asarray(Wv))
    res = run_bass_kernel_spmd(nc, in_maps, core_ids=list(range(8)), trace=False)
    out = np.empty((B, T, H), np.float32)
    for c in range(8):
        b, p = c // 2, c % 2
        # out_d[i, v, :] = row 128v + i; chunk m = rows 512m..512m+512
        r = res.results[c]["out"].reshape(128, 16, H).transpose(1, 0, 2)
        for m in range(NCH):
            out[b, CH * (2 * m + p) : CH * (2 * m + p + 1), :] = (
                r[4 * m : 4 * m + 4].reshape(CH, H)
            )
    return out
